# revision 25
# baseline (speedup 1.0000x reference)
"""Trainium2 Bass kernel for nn_Net_79465484911206: GRU(H=8) over x[4096,200,64] -> [4096].

Data parallel across 8 cores (512 samples each, as 4 chunks of 128 on
partitions).  The per-step dependency cycle is the bottleneck (engines are
~75% idle), so the kernel is organized to minimize the serial chain:

- h' = (1-z)*n + z*h is never materialized on the critical path.  The
  recurrent matmul W_hh @ h' is split into W_hh @ zh (ready early, off the
  chain) and W_hh @ ny (the only late dependency), using transposed bf16
  stationaries zhT / nyT.
- PSUM is split per gate: ps_r (r pre-acts), ps_z (z pre-acts), and ps_nx
  with ghn/xpn INTERLEAVED ([..., 0] = W_hn h part, [..., 1] = x_n proj).
  The n-gate input u = r*ghn + xpn is then ONE tensor_tensor_scan over the
  interleaved layout: d0 = [0 | r] resets the scan state at even slots
  (state = ghn), and odd slots produce r*ghn + xpn.  This removes a whole
  DVE->DVE handoff from the serial chain.
- The r-part recurrent matmuls are emitted first so sigmoid_r's wait
  resolves ~100ns before the z/n parts finish.
- sigmoid_z is dep-pinned behind sigmoid_r so the static scheduler cannot
  slot it onto the critical path.

Critical cycle per step:
  mm_ny_r (PE) -> sigmoid_r (ACT) -> scan_u (DVE) -> tanh (ACT)
  -> ny (DVE) -> nyT (DVE) -> mm_ny (PE, next step)

Self-contained: hardcodes all shapes; host does sharding + layout prep.
"""

import os
import numpy as np
import ml_dtypes

bf16 = ml_dtypes.bfloat16

B, T, F, H = 4096, 200, 64, 8
NCORES = 8
BL = B // NCORES          # 512 per core
NCH = 4                   # chunks of 128 samples
T2 = T // 2               # 100
DMA_T2 = 10               # t2-steps per x DMA chunk

LAST_RESULTS = None       # test.py reads exec_time_ns from here


def _build_program(b_dec_val: float):
    import concourse.bacc as bacc
    import concourse.mybir as mybir
    from concourse.tile import TileContext
    from concourse.tile_rust import add_dep_helper

    AF = mybir.ActivationFunctionType
    ALU = mybir.AluOpType
    dt = mybir.dt

    nc = bacc.Bacc(
        "TRN2", target_bir_lowering=False, debug=False, num_devices=NCORES
    )

    x2_d = nc.dram_tensor("x2", [128, T2, NCH, 128], dt.bfloat16, kind="ExternalInput").ap()
    w128_d = nc.dram_tensor("w128", [128, 152], dt.bfloat16, kind="ExternalInput").ap()
    rows_d = nc.dram_tensor("rows", [1, 256], dt.bfloat16, kind="ExternalInput").ap()
    out_d = nc.dram_tensor("out", [128, NCH], dt.float32, kind="ExternalOutput").ap()

    with TileContext(nc) as tc:
        with (
            tc.tile_pool(name="consts", bufs=1) as cpool,
            tc.tile_pool(name="xin", bufs=3) as xpool,
            tc.tile_pool(name="state", bufs=1) as spool,
            tc.tile_pool(name="work", bufs=3) as wpool,
            tc.tile_pool(name="psr", bufs=2, space="PSUM") as prpool,
            tc.tile_pool(name="psz", bufs=2, space="PSUM") as pzpool,
            tc.tile_pool(name="psnx", bufs=2, space="PSUM") as nxpool,
        ):
            # constants: two packed DMAs (one 128-row, one 1-row)
            w128 = cpool.tile([128, 152], dt.bfloat16, tag="w128", name="w128")
            nc.sync.dma_start(out=w128[:], in_=w128_d)
            rows = cpool.tile([1, 256], dt.bfloat16, tag="rows", name="rows")
            nc.sync.dma_start(out=rows[:], in_=rows_d)
            wihr = w128[:, 0:8]
            wihz = w128[:, 8:16]
            wihn = w128[:, 16:24]
            whhr = w128[:, 24:56]
            whhz = w128[:, 56:88]
            whhn = w128[:, 88:120]
            wdec = w128[:, 120:152]
            biasr = rows[:, 0:32]
            biasz = rows[:, 32:64]
            biasnx = rows[:, 64:128]
            ones = rows[:, 128:256]

            # state: h [128, (4, 8)] bf16; zhT/nyT transposed update parts;
            # rint = [0 | r] interleaved (even slots stay zero forever)
            h = spool.tile([128, 32], dt.bfloat16)
            nc.vector.memset(h[:], 0.0)
            zhT = spool.tile([128, 32], dt.bfloat16)
            nc.vector.memset(zhT[:], 0.0)
            nyT = spool.tile([128, 32], dt.bfloat16)
            nc.vector.memset(nyT[:], 0.0)
            rint = spool.tile([128, NCH, 8, 2], dt.bfloat16)
            nc.vector.memset(rint[:], 0.0)

            xsb = None
            xsb_map = {}
            ps_map = {}

            def emit_x(t):
                """bias + x matmuls for step t (everything h-independent)."""
                t2, tp = divmod(t, 2)
                ki, ko = t2_chunk[t2]
                xsb = xsb_map[ki]
                psr = prpool.tile([128, NCH, 8], dt.float32, tag="psr", name=f"psr{t}")
                psz = pzpool.tile([128, NCH, 8], dt.float32, tag="psz", name=f"psz{t}")
                psnx = nxpool.tile([128, NCH, 8, 2], dt.float32, tag="psnx", name=f"psnx{t}")
                psr2 = psr[:].rearrange("p c g -> p (c g)")
                psz2 = psz[:].rearrange("p c g -> p (c g)")
                psnx2 = psnx[:].rearrange("p c g two -> p (c g two)")
                b_r = nc.tensor.matmul(psr2, ones, biasr, start=True, stop=False,
                                       skip_group_check=True)
                b_z = nc.tensor.matmul(psz2, ones, biasz, start=True, stop=False,
                                       skip_group_check=True)
                b_nx = nc.tensor.matmul(psnx2, ones, biasnx, start=True, stop=False,
                                        skip_group_check=True)
                xs = []
                for c in range(NCH):
                    stat = xsb[tp * 64:(tp + 1) * 64, ko, c, :]
                    m_r = nc.tensor.matmul(psr[:, c, :], stat, wihr[tp * 64:(tp + 1) * 64, :],
                                           start=False, stop=False, skip_group_check=True)
                    m_z = nc.tensor.matmul(psz[:, c, :], stat, wihz[tp * 64:(tp + 1) * 64, :],
                                           start=False, stop=False, skip_group_check=True)
                    m_x = nc.tensor.matmul(psnx[:, c, :, 1], stat, wihn[tp * 64:(tp + 1) * 64, :],
                                           start=False, stop=False, skip_group_check=True)
                    add_dep_helper(m_r.ins, b_r.ins, False, "accum order")
                    add_dep_helper(m_z.ins, b_z.ins, False, "accum order")
                    add_dep_helper(m_x.ins, b_nx.ins, False, "accum order")
                    xs.append((m_r, m_z, m_x))
                ps_map[t] = (psr, psz, psnx, xs)

            def emit_hmm(t, statT, stop, after=None):
                """Block matmuls of W_hh against stationary statT (zhT or nyT).
                r-part first so sigmoid_r's wait resolves early."""
                psr, psz, psnx, xs = ps_map[t]
                psr2 = psr[:].rearrange("p c g -> p (c g)")
                psz2 = psz[:].rearrange("p c g -> p (c g)")
                out = []
                for part, mov in enumerate((whhr, whhz, whhn)):
                    for i in range(NCH):
                        last = stop and (i == NCH - 1)
                        if part == 0:
                            o = psr2[32 * i:32 * (i + 1), :]
                        elif part == 1:
                            o = psz2[32 * i:32 * (i + 1), :]
                        else:
                            o = psnx[32 * i:32 * (i + 1), :, :, 0]
                        mm = nc.tensor.matmul(
                            o, statT[32 * i:32 * (i + 1), :],
                            mov[32 * i:32 * (i + 1), :],
                            start=False, stop=last, skip_group_check=True,
                            tile_position=(32 * i, 32 * i))
                        out.append(mm)
                for k, mm in enumerate(out):
                    part, i = divmod(k, NCH)
                    for (m_r, m_z, m_x) in xs:
                        add_dep_helper(mm.ins, (m_r, m_z, m_x)[part].ins,
                                       False, "accum order")
                    if after is not None:
                        add_dep_helper(mm.ins, after[k].ins, False, "accum order")
                return out

            def emit_step(t):
                psr, psz, psnx, xs = ps_map[t]
                z = wpool.tile([128, NCH, 8], dt.bfloat16, tag="z", name="z")
                uu = wpool.tile([128, NCH, 8, 2], dt.float32, tag="uu", name="uu")
                n = wpool.tile([128, NCH, 8], dt.bfloat16, tag="n", name="n")
                y = wpool.tile([128, NCH, 8], dt.bfloat16, tag="y", name="y")
                zh = wpool.tile([128, NCH, 8], dt.bfloat16, tag="zh", name="zh")
                ny = wpool.tile([128, NCH, 8], dt.bfloat16, tag="ny", name="ny")

                # on-chain: sigmoid_r (strided out into rint odd slots)
                s_r = nc.scalar.activation(rint[:, :, :, 1], psr[:], AF.Sigmoid)
                # off-chain: sigmoid_z pinned behind sigmoid_r
                s_z = nc.scalar.activation(z[:], psz[:], AF.Sigmoid)
                add_dep_helper(s_z.ins, s_r.ins, False, "keep sig_z off chain")

                # on-chain: u = r*ghn + xpn in ONE scan op
                # (even slots: state resets to ghn; odd slots: r*state + xpn)
                nc.vector.tensor_tensor_scan(
                    uu[:].rearrange("p c g two -> p (c g two)"),
                    rint[:].rearrange("p c g two -> p (c g two)"),
                    psnx[:].rearrange("p c g two -> p (c g two)"),
                    0.0, ALU.mult, ALU.add)

                # on-chain: tanh from the odd (u) slots
                nc.scalar.activation(n[:], uu[:, :, :, 1], AF.Tanh)

                # off-chain (fills the scan->tanh window): y, zh, zhT
                nc.vector.tensor_scalar(y[:], z[:], -1.0, 1.0, ALU.mult, ALU.add)
                hv = h[:].rearrange("p (c j) -> p c j", c=NCH)
                nc.vector.tensor_mul(zh[:], z[:], hv)
                nc.vector.transpose(zhT[:], zh[:].rearrange("p c j -> p (c j)"))

                # on-chain tail: ny, nyT
                nc.vector.tensor_mul(ny[:], n[:], y[:])
                nc.vector.transpose(nyT[:], ny[:].rearrange("p c j -> p (c j)"))

                # off-chain: h' = ny + zh (for next zh and final decode)
                nc.vector.tensor_add(hv, ny[:], zh[:])

            # x DMA chunks: tiny first chunk so step 0 starts ASAP
            chunks = [(0, 2)] + [(2 + DMA_T2 * i, DMA_T2) for i in range(9)] + [(92, 8)]
            t2_chunk = {}
            for ci, (start, ln) in enumerate(chunks):
                for o in range(ln):
                    t2_chunk[start + o] = (ci, o)

            for t in range(T):
                t2, tp = divmod(t, 2)
                ki, ko = t2_chunk[t2]
                if ko == 0 and tp == 0:
                    start, ln = chunks[ki]
                    xsb = xpool.tile([128, ln, NCH, 128], dt.bfloat16,
                                     tag=f"xsb{ln}", name=f"xsb{ki}")
                    nc.sync.dma_start(
                        out=xsb[:],
                        in_=x2_d[:, start:start + ln, :, :],
                    )
                    xsb_map[ki] = xsb
                emit_x(t)
                # W_hh @ zh(t-1): ready early, off the critical chain
                # (at t=0 both stationaries are the zero-memset tiles: exact)
                zh_mms = emit_hmm(t, zhT, stop=False)
                # W_hh @ ny(t-1): the critical-path matmuls
                emit_hmm(t, nyT, stop=True, after=zh_mms)
                emit_step(t)
                ps_map.pop(t - 2, None)

            # decode: out[p, c] = sum_j h * wdec + b_dec
            prod = wpool.tile([128, NCH, 8], dt.float32, tag="prod")
            nc.vector.tensor_mul(
                prod[:], h[:].rearrange("p (c j) -> p c j", c=NCH),
                wdec.rearrange("p (c j) -> p c j", c=NCH),
            )
            res = wpool.tile([128, NCH, 1], dt.float32, tag="res")
            nc.vector.tensor_reduce(
                res[:], prod[:], axis=mybir.AxisListType.X, op=mybir.AluOpType.add
            )
            res2 = wpool.tile([128, NCH], dt.float32, tag="res2")
            nc.vector.tensor_scalar_add(
                res2[:], res[:].rearrange("p c one -> p (c one)"), float(b_dec_val))
            nc.sync.dma_start(out=out_d, in_=res2[:])

    nc.compile()
    return nc


def _prep_inputs(x, w_ih, w_hh, b_ih, b_hh, w_dec, b_dec):
    """Returns per-core in_maps list."""
    w_ih = np.asarray(w_ih, np.float32)
    w_hh = np.asarray(w_hh, np.float32)
    b_ih = np.asarray(b_ih, np.float32)
    b_hh = np.asarray(b_hh, np.float32)
    w_dec = np.asarray(w_dec, np.float32)

    # x-projection weights, stationary = x^T [64f, 128s], moving = wih*
    wihr = np.tile(w_ih[0:8].T, (2, 1)).astype(bf16)            # [128, 8]
    wihz = np.tile(w_ih[8:16].T, (2, 1)).astype(bf16)           # [128, 8]
    wihn = np.tile(w_ih[16:24].T, (2, 1)).astype(bf16)          # [128, 8]

    # recurrent weights, block-diag over chunks; stationary = (zh|ny)^T
    def blockdiag(wpart):
        # wpart: [8, 8] rows of w_hh ; returns [128, NCH*8]
        m = np.zeros((32, NCH, 8), np.float32)
        for c in range(NCH):
            m[c * 8:(c + 1) * 8, c, :] = wpart.T                # [8j, 8g]
        m = m.reshape(32, NCH * 8)
        return np.tile(m, (4, 1)).astype(bf16)

    whhr = blockdiag(w_hh[0:8])                                 # [128, 32]
    whhz = blockdiag(w_hh[8:16])                                # [128, 32]
    whhn = blockdiag(w_hh[16:24])                               # [128, 32]

    biasr = np.tile(b_ih[0:8] + b_hh[0:8], NCH).reshape(1, 32).astype(bf16)
    biasz = np.tile(b_ih[8:16] + b_hh[8:16], NCH).reshape(1, 32).astype(bf16)
    bnx = np.empty((NCH, 8, 2), np.float32)
    bnx[:, :, 0] = b_hh[16:24]                                  # ghn bias
    bnx[:, :, 1] = b_ih[16:24]                                  # xpn bias
    biasnx = bnx.reshape(1, 64).astype(bf16)

    ones = np.ones((1, 128), bf16)
    wdec_b = np.tile(w_dec[0].astype(bf16).astype(np.float32), (128, NCH)).astype(bf16)

    w128 = np.concatenate([wihr, wihz, wihn, whhr, whhz, whhn, wdec_b],
                          axis=1).astype(bf16)                  # [128, 152]
    rows = np.concatenate([biasr, biasz, biasnx, ones], axis=1).astype(bf16)  # [1, 256]

    x = np.asarray(x, np.float32)
    in_maps = []
    for core in range(NCORES):
        xc = x[core * BL:(core + 1) * BL]                      # [512, 200, 64]
        tmp = xc.reshape(NCH, 128, T2, 2, 64)                  # ch, s, t2, tp, f
        x2 = np.ascontiguousarray(
            tmp.transpose(3, 4, 2, 0, 1).reshape(128, T2, NCH, 128)
        ).astype(bf16)
        in_maps.append({"x2": x2, "w128": w128, "rows": rows})
    return in_maps


def kernel(x, w_ih, w_hh, b_ih, b_hh, w_dec, b_dec):
    global LAST_RESULTS
    from concourse import bass_utils

    b_dec_val = float(np.asarray(b_dec, np.float32).reshape(-1)[0])
    nc = _build_program(b_dec_val)
    in_maps = _prep_inputs(x, w_ih, w_hh, b_ih, b_hh, w_dec, b_dec)
    res = bass_utils.run_bass_kernel_spmd(
        nc, in_maps, core_ids=list(range(NCORES)),
        trace=bool(int(os.environ.get("KERNEL_TRACE", "0"))),
    )
    LAST_RESULTS = res
    out = np.empty(B, np.float32)
    for core in range(NCORES):
        o = np.asarray(res.results[core]["out"])               # [128, 4]
        out[core * BL:(core + 1) * BL] = o.T.reshape(-1)
    return out


# revision 26
# speedup vs baseline: 1.0009x; 1.0009x over previous
"""Trainium2 Bass kernel for nn_Net_79465484911206: GRU(H=8) over x[4096,200,64] -> [4096].

Data parallel across 8 cores (512 samples each, as 4 chunks of 128 on
partitions).  The per-step dependency cycle is the bottleneck (engines are
~75% idle), so the kernel is organized to minimize the serial chain:

- h' = (1-z)*n + z*h is never materialized on the critical path.  The
  recurrent matmul W_hh @ h' is split into W_hh @ zh (ready early, off the
  chain) and W_hh @ ny (the only late dependency), using transposed bf16
  stationaries zhT / nyT.
- PSUM is split per gate: ps_r (r pre-acts), ps_z (z pre-acts), and ps_nx
  with ghn/xpn INTERLEAVED ([..., 0] = W_hn h part, [..., 1] = x_n proj).
  The n-gate input u = r*ghn + xpn is then ONE tensor_tensor_scan over the
  interleaved layout: d0 = [0 | r] resets the scan state at even slots
  (state = ghn), and odd slots produce r*ghn + xpn.  This removes a whole
  DVE->DVE handoff from the serial chain.
- The r-part recurrent matmuls are emitted first so sigmoid_r's wait
  resolves ~100ns before the z/n parts finish.
- sigmoid_z is dep-pinned behind sigmoid_r so the static scheduler cannot
  slot it onto the critical path.

Critical cycle per step:
  mm_ny_r (PE) -> sigmoid_r (ACT) -> scan_u (DVE) -> tanh (ACT)
  -> ny (DVE) -> nyT (DVE) -> mm_ny (PE, next step)

Self-contained: hardcodes all shapes; host does sharding + layout prep.
"""

import os
import numpy as np
import ml_dtypes

bf16 = ml_dtypes.bfloat16

B, T, F, H = 4096, 200, 64, 8
NCORES = 8
BL = B // NCORES          # 512 per core
NCH = 4                   # chunks of 128 samples
T2 = T // 2               # 100
DMA_T2 = 10               # t2-steps per x DMA chunk

LAST_RESULTS = None       # test.py reads exec_time_ns from here


def _build_program(b_dec_val: float):
    import concourse.bacc as bacc
    import concourse.mybir as mybir
    from concourse.tile import TileContext
    from concourse.tile_rust import add_dep_helper

    AF = mybir.ActivationFunctionType
    ALU = mybir.AluOpType
    dt = mybir.dt

    nc = bacc.Bacc(
        "TRN2", target_bir_lowering=False, debug=False, num_devices=NCORES
    )

    x2_d = nc.dram_tensor("x2", [128, T2, NCH, 128], dt.bfloat16, kind="ExternalInput").ap()
    w128_d = nc.dram_tensor("w128", [128, 152], dt.bfloat16, kind="ExternalInput").ap()
    rows_d = nc.dram_tensor("rows", [1, 256], dt.bfloat16, kind="ExternalInput").ap()
    out_d = nc.dram_tensor("out", [128, NCH], dt.float32, kind="ExternalOutput").ap()

    with TileContext(nc) as tc:
        with (
            tc.tile_pool(name="consts", bufs=1) as cpool,
            tc.tile_pool(name="xin", bufs=3) as xpool,
            tc.tile_pool(name="state", bufs=1) as spool,
            tc.tile_pool(name="work", bufs=3) as wpool,
            tc.tile_pool(name="psr", bufs=2, space="PSUM") as prpool,
            tc.tile_pool(name="psz", bufs=2, space="PSUM") as pzpool,
            tc.tile_pool(name="psnx", bufs=2, space="PSUM") as nxpool,
        ):
            # first x chunk DMA issued before everything else: it gates step 0
            xsb0 = xpool.tile([128, 2, NCH, 128], dt.bfloat16, tag="xsb2", name="xsb0")
            nc.sync.dma_start(out=xsb0[:], in_=x2_d[:, 0:2, :, :])

            # constants: two packed DMAs (one 128-row, one 1-row)
            w128 = cpool.tile([128, 152], dt.bfloat16, tag="w128", name="w128")
            nc.sync.dma_start(out=w128[:], in_=w128_d)
            rows = cpool.tile([1, 256], dt.bfloat16, tag="rows", name="rows")
            nc.sync.dma_start(out=rows[:], in_=rows_d)
            wihr = w128[:, 0:8]
            wihz = w128[:, 8:16]
            wihn = w128[:, 16:24]
            whhr = w128[:, 24:56]
            whhz = w128[:, 56:88]
            whhn = w128[:, 88:120]
            wdec = w128[:, 120:152]
            biasr = rows[:, 0:32]
            biasz = rows[:, 32:64]
            biasnx = rows[:, 64:128]
            ones = rows[:, 128:256]

            # state: h [128, (4, 8)] bf16; zhT/nyT transposed update parts;
            # rint = [0 | r] interleaved (even slots stay zero forever)
            h = spool.tile([128, 32], dt.bfloat16)
            nc.vector.memset(h[:], 0.0)
            zhT = spool.tile([128, 32], dt.bfloat16)
            nc.vector.memset(zhT[:], 0.0)
            nyT = spool.tile([128, 32], dt.bfloat16)
            nc.vector.memset(nyT[:], 0.0)
            rint = spool.tile([128, NCH, 8, 2], dt.bfloat16)
            nc.vector.memset(rint[:], 0.0)

            xsb = None
            xsb_map = {}
            ps_map = {}

            def emit_x(t):
                """bias + x matmuls for step t (everything h-independent)."""
                t2, tp = divmod(t, 2)
                ki, ko = t2_chunk[t2]
                xsb = xsb_map[ki]
                psr = prpool.tile([128, NCH, 8], dt.float32, tag="psr", name=f"psr{t}")
                psz = pzpool.tile([128, NCH, 8], dt.float32, tag="psz", name=f"psz{t}")
                psnx = nxpool.tile([128, NCH, 8, 2], dt.float32, tag="psnx", name=f"psnx{t}")
                psr2 = psr[:].rearrange("p c g -> p (c g)")
                psz2 = psz[:].rearrange("p c g -> p (c g)")
                psnx2 = psnx[:].rearrange("p c g two -> p (c g two)")
                b_r = nc.tensor.matmul(psr2, ones, biasr, start=True, stop=False,
                                       skip_group_check=True)
                b_z = nc.tensor.matmul(psz2, ones, biasz, start=True, stop=False,
                                       skip_group_check=True)
                b_nx = nc.tensor.matmul(psnx2, ones, biasnx, start=True, stop=False,
                                        skip_group_check=True)
                xs = []
                for c in range(NCH):
                    stat = xsb[tp * 64:(tp + 1) * 64, ko, c, :]
                    m_r = nc.tensor.matmul(psr[:, c, :], stat, wihr[tp * 64:(tp + 1) * 64, :],
                                           start=False, stop=False, skip_group_check=True)
                    m_z = nc.tensor.matmul(psz[:, c, :], stat, wihz[tp * 64:(tp + 1) * 64, :],
                                           start=False, stop=False, skip_group_check=True)
                    m_x = nc.tensor.matmul(psnx[:, c, :, 1], stat, wihn[tp * 64:(tp + 1) * 64, :],
                                           start=False, stop=False, skip_group_check=True)
                    add_dep_helper(m_r.ins, b_r.ins, False, "accum order")
                    add_dep_helper(m_z.ins, b_z.ins, False, "accum order")
                    add_dep_helper(m_x.ins, b_nx.ins, False, "accum order")
                    xs.append((m_r, m_z, m_x))
                ps_map[t] = (psr, psz, psnx, xs)

            def emit_hmm(t, statT, stop, after=None):
                """Block matmuls of W_hh against stationary statT (zhT or nyT).
                r-part first so sigmoid_r's wait resolves early."""
                psr, psz, psnx, xs = ps_map[t]
                psr2 = psr[:].rearrange("p c g -> p (c g)")
                psz2 = psz[:].rearrange("p c g -> p (c g)")
                out = []
                for part, mov in enumerate((whhr, whhz, whhn)):
                    for i in range(NCH):
                        last = stop and (i == NCH - 1)
                        if part == 0:
                            o = psr2[32 * i:32 * (i + 1), :]
                        elif part == 1:
                            o = psz2[32 * i:32 * (i + 1), :]
                        else:
                            o = psnx[32 * i:32 * (i + 1), :, :, 0]
                        mm = nc.tensor.matmul(
                            o, statT[32 * i:32 * (i + 1), :],
                            mov[32 * i:32 * (i + 1), :],
                            start=False, stop=last, skip_group_check=True,
                            tile_position=(32 * i, 32 * i))
                        out.append(mm)
                for k, mm in enumerate(out):
                    part, i = divmod(k, NCH)
                    for (m_r, m_z, m_x) in xs:
                        add_dep_helper(mm.ins, (m_r, m_z, m_x)[part].ins,
                                       False, "accum order")
                    if after is not None:
                        add_dep_helper(mm.ins, after[k].ins, False, "accum order")
                return out

            def emit_step(t):
                psr, psz, psnx, xs = ps_map[t]
                z = wpool.tile([128, NCH, 8], dt.bfloat16, tag="z", name="z")
                uu = wpool.tile([128, NCH, 8, 2], dt.float32, tag="uu", name="uu")
                n = wpool.tile([128, NCH, 8], dt.bfloat16, tag="n", name="n")
                y = wpool.tile([128, NCH, 8], dt.bfloat16, tag="y", name="y")
                zh = wpool.tile([128, NCH, 8], dt.bfloat16, tag="zh", name="zh")
                ny = wpool.tile([128, NCH, 8], dt.bfloat16, tag="ny", name="ny")

                # on-chain: sigmoid_r (strided out into rint odd slots)
                s_r = nc.scalar.activation(rint[:, :, :, 1], psr[:], AF.Sigmoid)
                # off-chain: sigmoid_z pinned behind sigmoid_r
                s_z = nc.scalar.activation(z[:], psz[:], AF.Sigmoid)
                add_dep_helper(s_z.ins, s_r.ins, False, "keep sig_z off chain")

                # on-chain: u = r*ghn + xpn in ONE scan op
                # (even slots: state resets to ghn; odd slots: r*state + xpn)
                nc.vector.tensor_tensor_scan(
                    uu[:].rearrange("p c g two -> p (c g two)"),
                    rint[:].rearrange("p c g two -> p (c g two)"),
                    psnx[:].rearrange("p c g two -> p (c g two)"),
                    0.0, ALU.mult, ALU.add)

                # on-chain: tanh from the odd (u) slots
                nc.scalar.activation(n[:], uu[:, :, :, 1], AF.Tanh)

                # off-chain (fills the scan->tanh window): y, zh, zhT
                nc.vector.tensor_scalar(y[:], z[:], -1.0, 1.0, ALU.mult, ALU.add)
                hv = h[:].rearrange("p (c j) -> p c j", c=NCH)
                nc.vector.tensor_mul(zh[:], z[:], hv)
                nc.vector.transpose(zhT[:], zh[:].rearrange("p c j -> p (c j)"))

                # on-chain tail: ny, nyT
                nc.vector.tensor_mul(ny[:], n[:], y[:])
                nc.vector.transpose(nyT[:], ny[:].rearrange("p c j -> p (c j)"))

                # off-chain: h' = ny + zh (for next zh and final decode)
                nc.vector.tensor_add(hv, ny[:], zh[:])

            # x DMA chunks: tiny first chunk so step 0 starts ASAP
            chunks = [(0, 2)] + [(2 + DMA_T2 * i, DMA_T2) for i in range(9)] + [(92, 8)]
            t2_chunk = {}
            for ci, (start, ln) in enumerate(chunks):
                for o in range(ln):
                    t2_chunk[start + o] = (ci, o)

            xsb_map[0] = xsb0
            for t in range(T):
                t2, tp = divmod(t, 2)
                ki, ko = t2_chunk[t2]
                if ko == 0 and tp == 0 and ki > 0:
                    start, ln = chunks[ki]
                    xsb = xpool.tile([128, ln, NCH, 128], dt.bfloat16,
                                     tag=f"xsb{ln}", name=f"xsb{ki}")
                    nc.sync.dma_start(
                        out=xsb[:],
                        in_=x2_d[:, start:start + ln, :, :],
                    )
                    xsb_map[ki] = xsb
                emit_x(t)
                # W_hh @ zh(t-1): ready early, off the critical chain
                # (at t=0 both stationaries are the zero-memset tiles: exact)
                zh_mms = emit_hmm(t, zhT, stop=False)
                # W_hh @ ny(t-1): the critical-path matmuls
                emit_hmm(t, nyT, stop=True, after=zh_mms)
                emit_step(t)
                ps_map.pop(t - 2, None)

            # decode: out[p, c] = sum_j h * wdec + b_dec
            prod = wpool.tile([128, NCH, 8], dt.float32, tag="prod")
            nc.vector.tensor_mul(
                prod[:], h[:].rearrange("p (c j) -> p c j", c=NCH),
                wdec.rearrange("p (c j) -> p c j", c=NCH),
            )
            res = wpool.tile([128, NCH, 1], dt.float32, tag="res")
            nc.vector.tensor_reduce(
                res[:], prod[:], axis=mybir.AxisListType.X, op=mybir.AluOpType.add
            )
            res2 = wpool.tile([128, NCH], dt.float32, tag="res2")
            nc.vector.tensor_scalar_add(
                res2[:], res[:].rearrange("p c one -> p (c one)"), float(b_dec_val))
            nc.sync.dma_start(out=out_d, in_=res2[:])

    nc.compile()
    return nc


def _prep_inputs(x, w_ih, w_hh, b_ih, b_hh, w_dec, b_dec):
    """Returns per-core in_maps list."""
    w_ih = np.asarray(w_ih, np.float32)
    w_hh = np.asarray(w_hh, np.float32)
    b_ih = np.asarray(b_ih, np.float32)
    b_hh = np.asarray(b_hh, np.float32)
    w_dec = np.asarray(w_dec, np.float32)

    # x-projection weights, stationary = x^T [64f, 128s], moving = wih*
    wihr = np.tile(w_ih[0:8].T, (2, 1)).astype(bf16)            # [128, 8]
    wihz = np.tile(w_ih[8:16].T, (2, 1)).astype(bf16)           # [128, 8]
    wihn = np.tile(w_ih[16:24].T, (2, 1)).astype(bf16)          # [128, 8]

    # recurrent weights, block-diag over chunks; stationary = (zh|ny)^T
    def blockdiag(wpart):
        # wpart: [8, 8] rows of w_hh ; returns [128, NCH*8]
        m = np.zeros((32, NCH, 8), np.float32)
        for c in range(NCH):
            m[c * 8:(c + 1) * 8, c, :] = wpart.T                # [8j, 8g]
        m = m.reshape(32, NCH * 8)
        return np.tile(m, (4, 1)).astype(bf16)

    whhr = blockdiag(w_hh[0:8])                                 # [128, 32]
    whhz = blockdiag(w_hh[8:16])                                # [128, 32]
    whhn = blockdiag(w_hh[16:24])                               # [128, 32]

    biasr = np.tile(b_ih[0:8] + b_hh[0:8], NCH).reshape(1, 32).astype(bf16)
    biasz = np.tile(b_ih[8:16] + b_hh[8:16], NCH).reshape(1, 32).astype(bf16)
    bnx = np.empty((NCH, 8, 2), np.float32)
    bnx[:, :, 0] = b_hh[16:24]                                  # ghn bias
    bnx[:, :, 1] = b_ih[16:24]                                  # xpn bias
    biasnx = bnx.reshape(1, 64).astype(bf16)

    ones = np.ones((1, 128), bf16)
    wdec_b = np.tile(w_dec[0].astype(bf16).astype(np.float32), (128, NCH)).astype(bf16)

    w128 = np.concatenate([wihr, wihz, wihn, whhr, whhz, whhn, wdec_b],
                          axis=1).astype(bf16)                  # [128, 152]
    rows = np.concatenate([biasr, biasz, biasnx, ones], axis=1).astype(bf16)  # [1, 256]

    x = np.asarray(x, np.float32)
    in_maps = []
    for core in range(NCORES):
        xc = x[core * BL:(core + 1) * BL]                      # [512, 200, 64]
        tmp = xc.reshape(NCH, 128, T2, 2, 64)                  # ch, s, t2, tp, f
        x2 = np.ascontiguousarray(
            tmp.transpose(3, 4, 2, 0, 1).reshape(128, T2, NCH, 128)
        ).astype(bf16)
        in_maps.append({"x2": x2, "w128": w128, "rows": rows})
    return in_maps


def kernel(x, w_ih, w_hh, b_ih, b_hh, w_dec, b_dec):
    global LAST_RESULTS
    from concourse import bass_utils

    b_dec_val = float(np.asarray(b_dec, np.float32).reshape(-1)[0])
    nc = _build_program(b_dec_val)
    in_maps = _prep_inputs(x, w_ih, w_hh, b_ih, b_hh, w_dec, b_dec)
    res = bass_utils.run_bass_kernel_spmd(
        nc, in_maps, core_ids=list(range(NCORES)),
        trace=bool(int(os.environ.get("KERNEL_TRACE", "0"))),
    )
    LAST_RESULTS = res
    out = np.empty(B, np.float32)
    for core in range(NCORES):
        o = np.asarray(res.results[core]["out"])               # [128, 4]
        out[core * BL:(core + 1) * BL] = o.T.reshape(-1)
    return out
